# revision 9
# baseline (speedup 1.0000x reference)
"""Multi-head attention Bass kernel v3 for Trainium2, SPMD over 8 NeuronCores.

Problem: q,k,v [4, 16, 2048, 64] fp32 -> softmax(q@k^T/sqrt(64))@v.
64 (batch*head) heads, 8 consecutive heads per core, no cross-core
communication.

Host precasts inputs to f16 (q,k prescaled by 2^-4 each, exact powers of
two) AND pre-packs the transposed layouts the device wants, so the device
does plain contiguous-ish DMAs (no xbar-transpose, no on-device k-swap):

  qt/kt [128, 1024]: partitions 0:64 = d of even n, 64:128 = d of odd n,
  col c = n-pair c.  kw = kt with partition halves swapped (so every
  (q-parity, k-parity) S matmul finds lhsT and rhs on the same partition
  range).  vt [128, 16, 64] feeds the vaug [128, 16*65] tile whose col 64
  per 65-block is memset to 1.0 so the PV matmul also accumulates the
  softmax denominator.

Per-core per-head (N=2048, D=64), 32 S-steps (q-parity x 16 k-tiles):
  - S^T tile [128 k, 1024 q] f32 PSUM (2 f16 matmuls, contraction d=64).
  - exp(32*s) split across THREE engines per a static plan:
      ACT: table exp, ~0.94us/step (most steps)
      DVE: 2-op custom cubic(z/32)^32 (pass B emission deferred 2 steps so
           pass A of later steps keeps the S-psum ring moving)
      POOL: DVE 1-op cubic(z/16)^4 -> q^4 f16, then 2 GPSIMD f16 squarings
           -> q^16.  Slow (~4us) but off the critical engines; those PVs
           defer ~10 steps.
  - PV natural orientation: O[q-tile] [128, 65] += P^T-block.T @ vaug-block,
    f32 PSUM over the 16 k-tiles.  PV emission order only pins kt=0 first
    (start=True zeroes the bank) and kt=15 last (stop); middle k-tiles land
    whenever their exp engine delivers.
  - Drain: psum->sbuf copy split DVE/ACT (130 cols each), then the RAW
    [128, 260] (4 q-tiles x (64 cols + denominator)) is DMA'd out;
    normalization (num/denom) happens on the HOST, eliminating the on-device
    normalize pass entirely.

PSUM: 3x[128,1024] S ring (6 banks) + 2 O banks.
"""

import numpy as np

B, H, N, D = 4, 16, 2048, 64
NCORES = 8
HEADS = B * H          # 64
HPC = HEADS // NCORES  # 8 heads per core
NT = 16                # k tiles of 128 rows (0-7 even k, 8-15 odd k)
NSTEP = 32             # steps per head = 2 parities x 16 k-tiles
ACT_SCALE = 32.0

_CACHE = {}

# z/32 scheme (DVE 2-op): exp(32 s) = q(s)^32, cubic minimax on |s|<=7.2/32.
Z32_C = (1.0000400173833472, 0.5014175146307196, 0.16555244796209398)
# z/16 scheme (POOL): exp(32 s) = q(s)^16 with q ~ e^{2s}: cubic minimax for
# e^v on |v|<=0.45 with v=2s folded into the coefficients (x2, x4, x8).
Z16_C = (2.0011006099212336, 2.0277839111113912, 1.3135423890325262)

# per-(kt) exp-engine plan, same for both parities; entries are column
# segments [(kind, c0, c1), ...] covering 0:1024.  kt 0 and 15 are always
# ACT (fast single op; they gate the O-bank start/stop batches).
POOLKT = (1, 6)
DVEKT = (4, 9, 13)
SPLITKT = {11: (("pool", 0, 256), ("act", 256, 896), ("dve", 896, 1024))}
# PV due lags (steps) per exp kind; kt0/kt15 handled specially.
LAG = {"act": 2, "dve": 6, "pool": 10}
DVE_B_DELAY = 2   # pass-B emission deferral (steps)


# last parity of the last head: exp work that finishes promptly after the
# final S matmul (no late DVE/pool chains gating the stop batch + drain)
LP_POOLKT = (1, 5)
LP_DVEKT = (3, 8)


def _plan(kt, last_parity=False):
    poolkt = LP_POOLKT if last_parity else POOLKT
    dvekt = LP_DVEKT if last_parity else DVEKT
    if kt in poolkt:
        return (("pool", 0, 1024),)
    if kt in dvekt:
        return (("dve", 0, 1024),)
    if not last_parity and kt in SPLITKT:
        return SPLITKT[kt]
    return (("act", 0, 1024),)


def _register_dve_exp():
    """Register the two custom DVE ops (cubic+2 squarings; 3 squarings).
    TRN2 DVE = v3: 8 ALU stages per pass.  Coefficients are call-time
    scalars, so the same pass-A op serves both the z/32 and z/16 schemes."""
    if "dve_ops" in _CACHE:
        return _CACHE["dve_ops"]
    import concourse.dve_ops as dops
    from concourse.dve_ops import DveOp
    from concourse.dve_spec import Spec, Src0, C0, C1, C2, One, sq
    from concourse.dve_uop import DveOpSpec
    from concourse.dve_spec import lower, _has_src1 as has_src1
    import numpy as np_

    def _ref_expa(in0, in1, c0, c1, c2):
        f = np_.float32
        u = in0.astype(f)
        q = (f(1.0) + u * (f(c0) + u * (f(c1) + u * f(c2)))).astype(f)
        q = (q * q).astype(f)
        return (q * q).astype(f)

    def _ref_expb(in0, in1, c0, c1, c2):
        f = np_.float32
        q = (in0.astype(f) * in0.astype(f)).astype(f)
        q = (q * q).astype(f)
        return (q * q).astype(f)

    body_a = sq(sq(One + Src0 * (C0 + Src0 * (C1 + Src0 * C2))))
    body_b = sq(sq(sq(Src0)))
    spec_a = Spec(body=body_a, reference=_ref_expa)
    spec_b = Spec(body=body_b, reference=_ref_expb)

    ops = []
    for name, spec in (("EXP2A_MHA", spec_a), ("EXP2B_MHA", spec_b)):
        if name in dops._SUB_OPCODE_FOR_NAME:
            op = next(o for o in dops.OPS if o.name == name)
            ops.append(op)
            continue
        row = max(dops._SUB_OPCODE_FOR_NAME.values()) + 1
        assert row < 0x20
        dops._SUB_OPCODE_FOR_NAME[name] = row
        shas = {}
        for ver in ("v3", "v4"):
            try:
                spec_obj = DveOpSpec(name=name, opcode=row,
                                     uops=lower(spec, ver=ver),
                                     rd1_en=has_src1(spec))
                shas[ver] = spec_obj.sha(ver)
            except Exception:
                pass
        op = DveOp(name, spec, subdim=False, uops_sha=shas)
        dops.OPS.append(op)
        dops.CUSTOM_DVE_SPECS[name] = op.spec
        ops.append(op)
    _CACHE["dve_ops"] = ops
    return ops


def _build(reps=1):
    import concourse.tile as tile
    from concourse import bacc, mybir

    f32 = mybir.dt.float32
    f16 = mybir.dt.float16
    Exp = mybir.ActivationFunctionType.Exp

    nc = bacc.Bacc("TRN2", target_bir_lowering=False, debug=False,
                   num_devices=NCORES)
    qt_d = nc.dram_tensor("qt", [HPC, 128, 1024], f16,
                          kind="ExternalInput").ap()
    kt_d = nc.dram_tensor("kt", [HPC, 128, 1024], f16,
                          kind="ExternalInput").ap()
    kw_d = nc.dram_tensor("kw", [HPC, 128, 1024], f16,
                          kind="ExternalInput").ap()
    vt_d = nc.dram_tensor("vt", [HPC, 128, NT, 64], f16,
                          kind="ExternalInput").ap()
    o_d = nc.dram_tensor("o", [HPC, 2, 2, 128, 260], f32,
                         kind="ExternalOutput").ap()

    expa, expb = _register_dve_exp()

    with tile.TileContext(nc) as tc:
        with (
            tc.tile_pool(name="qsp", bufs=2) as qpool,
            tc.tile_pool(name="ksp", bufs=2) as kpool,
            tc.tile_pool(name="ksw", bufs=2) as wpool,
            tc.tile_pool(name="vap", bufs=3) as vpool,
            tc.tile_pool(name="pt", bufs=12) as ppool,
            tc.tile_pool(name="et", bufs=6) as epool,
            tc.tile_pool(name="pm", bufs=2) as mpool,
            tc.tile_pool(name="osb", bufs=3) as bpool,
            tc.tile_pool(name="spsum", bufs=3, space="PSUM") as spool,
            tc.tile_pool(name="opsum", bufs=1, space="PSUM") as opool,
        ):
            def emit_in_dmas(h, first=False):
                """Issue all input DMAs for head h; returns its tiles.  For
                the very first head the loads are chunked and spread across
                the three HWDGE queues (SP/ACT/DVE) so the first S matmul is
                gated only by a 128-col k chunk + a 512-col q half."""
                qsp = qpool.tile([128, 1024], f16, tag="qsp", name="qsp")
                ksp = kpool.tile([128, 1024], f16, tag="ksp", name="ksp")
                ksw = wpool.tile([128, 1024], f16, tag="ksw", name="ksw")
                vaug = vpool.tile([128, NT * 65], f16, tag="vaug", name="vaug")
                v3 = vaug.rearrange("p (t c) -> p t c", c=65)
                if first:
                    nc.sync.dma_start(ksp[:, 0:128], kt_d[h][:, 0:128])
                    nc.sync.dma_start(qsp[:, 0:512], qt_d[h][:, 0:512])
                    nc.scalar.dma_start(qsp[:, 512:1024],
                                        qt_d[h][:, 512:1024])
                    nc.scalar.dma_start(ksp[:, 128:512], kt_d[h][:, 128:512])
                    nc.gpsimd.dma_start(ksp[:, 512:1024],
                                        kt_d[h][:, 512:1024])
                    nc.gpsimd.dma_start(v3[:, :, 0:64], vt_d[h])
                    nc.gpsimd.dma_start(ksw, kw_d[h])
                else:
                    nc.sync.dma_start(qsp, qt_d[h])
                    nc.sync.dma_start(ksp, kt_d[h])
                    nc.sync.dma_start(v3[:, :, 0:64], vt_d[h])
                    nc.sync.dma_start(ksw, kw_d[h])
                nc.gpsimd.memset(v3[:, :, 64], 1.0)
                return {"qsp": qsp, "ksp": ksp, "ksw": ksw, "vaug": vaug}

            def lhs_k(t, qh, kt):
                """lhsT [64, 128] for k-tile kt at q-parity qh's range."""
                if kt < 8:   # even k tile
                    if qh == 0:
                        return t["ksp"][0:64, 128 * kt:128 * kt + 128]
                    return t["ksw"][64:128, 128 * kt:128 * kt + 128]
                kk = kt - 8
                if qh == 0:
                    return t["ksw"][0:64, 128 * kk:128 * kk + 128]
                return t["ksp"][64:128, 128 * kk:128 * kk + 128]

            def emit_s(t, gst, st, sq, bq, last_parity=False):
                qh, kt = divmod(st, NT)
                sT = spool.tile([128, 1024], f32, tag="sT", name="sT")
                w = lhs_k(t, qh, kt)
                q0 = 64 * qh
                for c in range(2):
                    nc.tensor.matmul(
                        sT[:, 512 * c:512 * c + 512], w,
                        t["qsp"][q0:q0 + 64, 512 * c:512 * c + 512],
                        start=True, stop=True)
                pT = ppool.tile([128, 1024], f16, tag="pT", name="pT")
                plan = (("act", 0, 1024),) if kt in (0, 15) \
                    else _plan(kt, last_parity)
                et = None
                for kind, a, b in plan:
                    if kind == "act":
                        nc.scalar.activation(pT[:, a:b], sT[:, a:b], Exp,
                                             scale=ACT_SCALE)
                    elif kind == "dve":
                        if et is None:
                            et = epool.tile([128, 1024], f16, tag="et",
                                            name="et")
                        nc.vector._custom_dve(expa, out=et[:, a:b],
                                              in0=sT[:, a:b], s0=Z32_C[0],
                                              s1=Z32_C[1], imm2=Z32_C[2])
                        bq.append((gst + DVE_B_DELAY, et[:, a:b], pT[:, a:b]))
                    else:  # pool
                        if et is None:
                            et = epool.tile([128, 1024], f16, tag="et",
                                            name="et")
                        nc.vector._custom_dve(expa, out=et[:, a:b],
                                              in0=sT[:, a:b], s0=Z16_C[0],
                                              s1=Z16_C[1], imm2=Z16_C[2])
                        m1 = mpool.tile([128, 1024], f16, tag="pm1",
                                        name="pm1")
                        nc.gpsimd.tensor_mul(m1[:, a:b], et[:, a:b],
                                             et[:, a:b])
                        nc.gpsimd.tensor_mul(pT[:, a:b], m1[:, a:b],
                                             m1[:, a:b])
                sq[st] = [pT, 2]

            def emit_pv(t, st, half, sq, octx):
                qh, kt = divmod(st, NT)
                if kt == 0:
                    octx[("o", half)] = opool.tile(
                        [128, 512], f32, tag=f"o{half}", name=f"o{half}")
                ob = octx[("o", half)]
                ent = sq[st]
                pT = ent[0]
                for j in range(4):
                    qt = 4 * half + j
                    # start=True zeroes the ENTIRE psum bank, so only the
                    # bank's very first matmul may set it
                    nc.tensor.matmul(
                        ob[:, 65 * j:65 * j + 65],
                        pT[:, 128 * qt:128 * qt + 128],
                        t["vaug"][:, 65 * kt:65 * kt + 65],
                        start=(kt == 0 and j == 0), stop=(kt == NT - 1))
                ent[1] -= 1
                if ent[1] == 0:
                    del sq[st]

            def emit_drain(octx, h, qh, half, last=False):
                """Copy one O bank psum->sbuf (split DVE/ACT) and DMA the
                raw numerator+denominator block out; host normalizes."""
                osb = bpool.tile([128, 260], f32, tag="osb", name="osb")
                ob = octx.pop(("o", half))
                nc.vector.tensor_copy(osb[:, 0:130], ob[:, 0:130])
                nc.scalar.copy(osb[:, 130:260], ob[:, 130:260])
                eng = nc.scalar if (last and half == 1) else nc.sync
                eng.dma_start(o_d[h][qh][half], osb)

            seq = [i % HPC for i in range(HPC * reps)]
            total = len(seq) * NSTEP

            # Warm the ACT exp table during the initial DMA fill.
            warm = bpool.tile([128, 1], f32, tag="warm", name="warm")
            nc.gpsimd.memset(warm, 0.0)
            warm_o = bpool.tile([128, 1], f16, tag="warmo", name="warmo")
            nc.scalar.activation(warm_o, warm, Exp, scale=1.0)

            tiles = {0: emit_in_dmas(seq[0], first=True)}
            pvq = []    # (due, order, hi, st, half)
            bq = []     # (due, et_slice, pT_slice) deferred DVE pass-B
            dq = []     # (due, hi, qh, half, last)
            sqs = {}    # hi -> {st: [pT, refcount]}
            octxs = {}  # hi -> {("o", half): tile}
            kt0_due = {}   # (hi, qh, half) -> due of kt0 PV
            max_due = {}   # (hi, qh, half) -> max due emitted
            prev15_due = [-10.0, -10.0]  # previous parity's kt15 due, per half
            order = 0

            def handle_pv(gst, phi, pst, phalf):
                emit_pv(tiles[phi], pst, phalf, sqs[phi],
                        octxs.setdefault(phi, {}))
                pqh, pkt = divmod(pst, NT)
                if pkt == NT - 1:
                    last = (phi == len(seq) - 1 and pqh == 1)
                    if last:
                        dq.append((gst + 0.3, phi, pqh, phalf, True))
                    else:
                        dq.append((gst + 1 + phalf, phi, pqh, phalf, False))
                if pst == NSTEP - 1 and phalf == 1:
                    tiles.pop(phi - 1, None)

            gst = 0
            while gst < total + 16:
                while pvq and pvq[0][0] <= gst - 0.5:
                    _, _, phi, pst, phalf = pvq.pop(0)
                    handle_pv(gst, phi, pst, phalf)
                while dq and dq[0][0] <= gst:
                    _, phi, pqh, phalf, plast = dq.pop(0)
                    emit_drain(octxs[phi], seq[phi], pqh, phalf, last=plast)
                while bq and bq[0][0] <= gst:
                    _, et_sl, pT_sl = bq.pop(0)
                    nc.vector._custom_dve(expb, out=pT_sl, in0=et_sl)
                if gst < total:
                    hi, st = divmod(gst, NSTEP)
                    qh, kt = divmod(st, NT)
                    lp = (hi == len(seq) - 1 and qh == 1)
                    sq = sqs.setdefault(hi, {})
                    emit_s(tiles[hi], gst, st, sq, bq, last_parity=lp)
                    if kt == 0:
                        # the previous parity's O-bank drain (scheduled at
                        # ceil(kt15 due)+1+half) must be emitted before this
                        # parity's start=True PV reuses the bank
                        import math
                        dues = tuple(
                            max(gst + 4 + hf,
                                math.ceil(prev15_due[hf]) + 2.25 + hf)
                            for hf in range(2))
                        for hf in range(2):
                            kt0_due[(hi, qh, hf)] = dues[hf]
                            max_due[(hi, qh, hf)] = dues[hf]
                    elif kt == NT - 1:
                        dues = tuple(
                            max(gst + 1.5, max_due[(hi, qh, hf)] + 0.25)
                            for hf in range(2))
                        for hf in range(2):
                            prev15_due[hf] = dues[hf]
                    else:
                        plan = _plan(kt, lp)
                        lags = [0.0, 0.0]
                        for kind, a, b in plan:
                            if a < 512:
                                lags[0] = max(lags[0], LAG[kind])
                            if b > 512:
                                lags[1] = max(lags[1], LAG[kind])
                        dues = tuple(
                            max(gst + lags[hf],
                                kt0_due[(hi, qh, hf)] + 0.25)
                            for hf in range(2))
                    for hf in range(2):
                        max_due[(hi, qh, hf)] = max(max_due[(hi, qh, hf)],
                                                    dues[hf])
                        pvq.append((dues[hf], order, hi, st, hf))
                        order += 1
                    pvq.sort()
                    if st == 2 and hi + 1 < len(seq):
                        tiles[hi + 1] = emit_in_dmas(seq[hi + 1])
                while pvq and pvq[0][0] <= gst:
                    _, _, phi, pst, phalf = pvq.pop(0)
                    handle_pv(gst, phi, pst, phalf)
                if gst >= total:
                    while dq:
                        _, phi, pqh, phalf, plast = dq.pop(0)
                        emit_drain(octxs[phi], seq[phi], pqh, phalf,
                                   last=plast)
                gst += 1

    nc.compile()
    return nc


def get_nc(reps=1):
    key = f"nc{reps}"
    if key not in _CACHE:
        _CACHE[key] = _build(reps)
    return _CACHE[key]


def _pack_inputs(q, k, v):
    """Host-side cast + layout packing (see module docstring)."""
    qf = (np.asarray(q, dtype=np.float32) * np.float32(2.0 ** -4)) \
        .astype(np.float16).reshape(HEADS, N, D)
    kf = (np.asarray(k, dtype=np.float32) * np.float32(2.0 ** -4)) \
        .astype(np.float16).reshape(HEADS, N, D)
    vf = np.asarray(v, dtype=np.float32).astype(np.float16) \
        .reshape(HEADS, N, D)
    # [h, n, d] -> [h, 128, 1024]: partition = (n&1)*64 + d, col = n//2
    qt = qf.reshape(HEADS, 1024, 2, 64).transpose(0, 2, 3, 1) \
        .reshape(HEADS, 128, 1024)
    kt = kf.reshape(HEADS, 1024, 2, 64).transpose(0, 2, 3, 1) \
        .reshape(HEADS, 128, 1024)
    kw = kt.reshape(HEADS, 2, 64, 1024)[:, ::-1].reshape(HEADS, 128, 1024)
    # v rows n = t*256 + p*2 + two -> vt[h, p, two*8 + t, d]
    vt = vf.reshape(HEADS, 8, 128, 2, 64).transpose(0, 2, 3, 1, 4) \
        .reshape(HEADS, 128, NT, 64)
    c = np.ascontiguousarray
    return [
        {"qt": c(qt[i * HPC:(i + 1) * HPC]),
         "kt": c(kt[i * HPC:(i + 1) * HPC]),
         "kw": c(kw[i * HPC:(i + 1) * HPC]),
         "vt": c(vt[i * HPC:(i + 1) * HPC])}
        for i in range(NCORES)
    ]


def _unpack_output(raws):
    """raws: NCORES x [HPC, 2, 2, 128, 260] f32 raw numerator+denominator.
    Host divide + relayout to [B, H, N, D]."""
    raw = np.concatenate(raws, axis=0)  # [HEADS, 2, 2, 128, 260]
    rr = raw.reshape(HEADS, 2, 2, 128, 4, 65)
    num = rr[..., 0:64]
    den = rr[..., 64:65]
    res = num / den  # [h, qh, half, p, j, d]
    # n = (half*4 + j)*256 + p*2 + qh
    out = res.transpose(0, 2, 4, 3, 1, 5).reshape(HEADS, N, D)
    return np.ascontiguousarray(out.reshape(B, H, N, D).astype(np.float32))


def kernel(q, k, v):
    from concourse.bass_utils import run_bass_kernel_spmd

    nc = get_nc()
    in_maps = _pack_inputs(q, k, v)
    res = run_bass_kernel_spmd(nc, in_maps, list(range(NCORES)))
    return _unpack_output([res.results[c]["o"] for c in range(NCORES)])
